# revision 13
# baseline (speedup 1.0000x reference)
"""GQA attention block (Wq/Wk/Wv -> RoPE -> softmax(QK^T)V -> Wo) on 8 Trainium2
NeuronCores.

Sharding (tensor-parallel per the head-sharding scheme):
  core c in 0..7: batch b = c // 4, head-group g = c % 4.
  Each core owns 8 q-heads (global 8g..8g+7) and 2 kv-heads (2g, 2g+1) of one
  batch element, computes its slice of q/k/v projections, RoPE, attention, and
  a partial o_proj (Wo rows for its heads). The all-reduce after o_proj is the
  host-side unshard: out[b] = sum of the 4 partial outputs of batch b.

On-device layout (per core), everything feature-on-partitions ("transposed"):
  xt    [D=2048, S=2048]   x^T for this batch
  QT    [E=512,  S]        q^T; partition-tile j holds head pair (j, j+4):
                           local head j (kv0) on partitions 0:64, head j+4
                           (kv1) on partitions 64:128. Wq columns are permuted
                           on the host to produce this layout directly.
  KT    [128, S]           k^T; kv0 on partitions 0:64, kv1 on 64:128.
  V     [S, 130] as 16 tiles [128, 130]: cols 0:64 v(kv0), col 64 ones,
                           cols 65:129 v(kv1), col 129 ones  (v_aug).
  scores^T per head: [sk, sq] so exp is ACT psum->sbuf and the attn@v
  contraction (over sk) uses v_aug as the stationary operand; row 64 of the
  attn@v output is the softmax denominator (ones column trick).

Matmuls run in bf16 (1.0 PE cycles/col vs fp32r's ~1.1, half the DMA/SBUF
footprint; ~5e-3 rel err vs the 2e-2 gate). PSUM accumulation stays fp32;
RoPE elementwise math and the softmax reciprocal stay fp32.
"""

import sys

if "/opt/trn_rl_repo" not in sys.path:
    sys.path.insert(0, "/opt/trn_rl_repo")

from contextlib import ExitStack

import ml_dtypes
import numpy as np

import concourse.bass as bass  # noqa: F401  (engine types via nc)
import concourse.tile as tile
from concourse import bacc, bass_utils, mybir

F32 = mybir.dt.float32
F32R = mybir.dt.float32r
BF16 = mybir.dt.bfloat16
AF = mybir.ActivationFunctionType
NP_BF16 = ml_dtypes.bfloat16

# Problem constants (hardcoded per harness contract)
B = 2
S = 2048  # sequence length
D = 2048  # d_model
N_HEADS = 32
N_KV = 8
HD = 64  # head dim
ROPE_BASE = 500000.0
N_CORES = 8

# Per-core derived
NQ = N_HEADS // 4  # 8 local q heads (4 head-groups)
E = NQ * HD  # 512 local q features
NPAIR = NQ // 2  # 4 head pairs / e-tiles
KVW = 2 * HD  # 128 local kv features
SC = 512  # s-chunk (projection + sq chunk)
NSC = S // SC  # 4
DT = D // 128  # 16 d-tiles
SKT = S // 128  # 16 sk tiles
ET = E // 128  # 4 e-tiles
SCALE = 1.0 / float(np.sqrt(HD))


def build_program():
    nc = bacc.Bacc(
        "TRN2", target_bir_lowering=False, debug=False, enable_asserts=False
    )

    xt = nc.dram_tensor("xt", [D, S], BF16, kind="ExternalInput").ap()
    wq = nc.dram_tensor("wq", [D, E], BF16, kind="ExternalInput").ap()
    wk = nc.dram_tensor("wk", [D, KVW], BF16, kind="ExternalInput").ap()
    wv = nc.dram_tensor("wv", [D, KVW], BF16, kind="ExternalInput").ap()
    wo = nc.dram_tensor("wo", [E, D], BF16, kind="ExternalInput").ap()
    cosd = nc.dram_tensor("cosd", [128, S], BF16, kind="ExternalInput").ap()
    sind = nc.dram_tensor("sind", [128, S], BF16, kind="ExternalInput").ap()
    rmat = nc.dram_tensor("rmat", [128, 128], F32R, kind="ExternalInput").ap()
    ident = nc.dram_tensor("ident", [128, 128], BF16, kind="ExternalInput").ap()
    ones1 = nc.dram_tensor("ones1", [1, 128], F32R, kind="ExternalInput").ap()
    onesc = nc.dram_tensor("onesc", [128, 1], BF16, kind="ExternalInput").ap()
    out = nc.dram_tensor("out", [S, D], F32, kind="ExternalOutput").ap()

    with tile.TileContext(nc) as tc, ExitStack() as ctx:
        persist = ctx.enter_context(tc.tile_pool(name="persist", bufs=1))

        # Persistent SBUF state
        qt_sb = [persist.tile([128, S], BF16, tag=f"qt{j}", name=f"qt{j}") for j in range(NPAIR)]
        kt_sb = persist.tile([128, S], BF16, tag="kt")
        v_sb = [persist.tile([128, 130], BF16, tag=f"v{j}", name=f"v{j}") for j in range(SKT)]
        ones1_sb = persist.tile([1, 128], F32R, tag="ones1")
        nc.sync.dma_start(out=ones1_sb, in_=ones1)
        onesc_sb = persist.tile([128, 1], BF16, tag="onesc")
        nc.sync.dma_start(out=onesc_sb, in_=onesc)

        # ---------------- Phase 1: projections + RoPE + V transpose -------------
        with (
            tc.tile_pool(name="xtp", bufs=2) as xtp,
            tc.tile_pool(name="wp", bufs=1) as wp,
            tc.tile_pool(name="ropec", bufs=1) as ropec,
            tc.tile_pool(name="p1st", bufs=3) as p1st,
            tc.tile_pool(name="qt_ps", bufs=2, space="PSUM") as qt_ps,
            tc.tile_pool(name="kv_ps", bufs=1, space="PSUM") as kv_ps,
            tc.tile_pool(name="rot_ps", bufs=2, space="PSUM") as rot_ps,
            tc.tile_pool(name="tr_ps", bufs=2, space="PSUM") as tr_ps,
        ):

            wq_r = wq.rearrange("(t p) e -> p t e", p=128)
            wk_r = wk.rearrange("(t p) e -> p t e", p=128)
            wv_r = wv.rearrange("(t p) e -> p t e", p=128)
            wq_sb = wp.tile([128, DT, E], BF16, tag="wq")
            wk_sb = wp.tile([128, DT, KVW], BF16, tag="wk")
            wv_sb = wp.tile([128, DT, KVW], BF16, tag="wv")
            xt_r = xt.rearrange("(t p) s -> p t s", p=128)
            xt_c0 = xtp.tile([128, DT, SC], BF16, tag="xt", name="xt_c0")
            rmat_sb = ropec.tile([128, 128], F32R, tag="rmat")
            nc.scalar.dma_start(out=rmat_sb, in_=rmat)
            ident_sb = ropec.tile([128, 128], BF16, tag="ident")
            nc.scalar.dma_start(out=ident_sb, in_=ident)
            cos_sb = ropec.tile([128, S], BF16, tag="cos")
            nc.gpsimd.dma_start(out=cos_sb, in_=cosd)
            sin_sb = ropec.tile([128, S], BF16, tag="sin")
            nc.gpsimd.dma_start(out=sin_sb, in_=sind)
            # Startup DMA is issue-latency-bound on one ring; spread chunk-0
            # xt across 4 engine HWDGE rings in 4-tile groups and batch the
            # small weight tensors into single transfers. wk first (gates the
            # K projection), wq/wv behind it (needed ~6us later).
            nc.sync.dma_start(out=wk_sb, in_=wk_r)
            xt_rings = [nc.sync, nc.scalar, nc.gpsimd, nc.sync]
            for g, eng in enumerate(xt_rings):
                eng.dma_start(
                    out=xt_c0[:, 4 * g : 4 * (g + 1), :],
                    in_=xt_r[:, 4 * g : 4 * (g + 1), 0:SC],
                )
            nc.gpsimd.dma_start(out=wv_sb, in_=wv_r)


            def rope(dst, src_ps, cs, raw_tag):
                """dst[:, cs*SC:+SC] = src_ps*cos + (R^T src_raw)*sin."""
                sl = bass.ts(cs, SC)
                raw = p1st.tile([128, SC], F32R, tag="raw", name="raw", bufs=3)
                nc.scalar.copy(raw, src_ps)
                rp = rot_ps.tile([128, SC], F32, tag="rot")
                nc.tensor.matmul(rp, rmat_sb, raw, start=True, stop=True)
                tcos = p1st.tile([128, SC], F32, tag="tmp", name="tcos", bufs=4)
                nc.vector.tensor_mul(tcos, raw, cos_sb[:, sl])
                tsin = p1st.tile([128, SC], F32, tag="tmp", name="tsin", bufs=4)
                nc.vector.tensor_mul(tsin, rp, sin_sb[:, sl])
                nc.vector.tensor_add(dst[:, sl], tcos, tsin)

            for cs in range(NSC):
                if cs == 0:
                    xt_t = xt_c0
                    # wq arrives while K/V of chunk 0 compute (two rings)
                    nc.sync.dma_start(out=wq_sb[:, 0:8, :], in_=wq_r[:, 0:8, :])
                    nc.scalar.dma_start(
                        out=wq_sb[:, 8:DT, :], in_=wq_r[:, 8:DT, :]
                    )
                else:
                    xt_t = xtp.tile([128, DT, SC], BF16, tag="xt")
                    for g, eng in enumerate(xt_rings):
                        eng.dma_start(
                            out=xt_t[:, 4 * g : 4 * (g + 1), :],
                            in_=xt_r[:, 4 * g : 4 * (g + 1), bass.ts(cs, SC)],
                        )

                # KT projection + rope
                kp = kv_ps.tile([128, SC], F32, tag="kt")
                for t in range(DT):
                    nc.tensor.matmul(
                        kp,
                        wk_sb[:, t, :],
                        xt_t[:, t, :],
                        start=(t == 0),
                        stop=(t == DT - 1),
                    )
                rope(kt_sb, kp, cs, "kraw")

                # V^T projection, then transpose 128-subtiles into v_sb
                vp = kv_ps.tile([128, SC], F32, tag="vt")
                for t in range(DT):
                    nc.tensor.matmul(
                        vp,
                        wv_sb[:, t, :],
                        xt_t[:, t, :],
                        start=(t == 0),
                        stop=(t == DT - 1),
                    )
                vt_sb = p1st.tile([128, SC], BF16, tag="vtsb", bufs=2)
                nc.vector.tensor_copy(vt_sb, vp)
                for ss in range(SC // 128):
                    sk = cs * (SC // 128) + ss
                    tp = tr_ps.tile([128, 128], BF16, tag="tr")
                    nc.tensor.transpose(tp, vt_sb[:, bass.ts(ss, 128)], ident_sb)
                    nc.vector.tensor_copy(v_sb[sk][:, 0:64], tp[:, 0:64])
                    nc.vector.tensor_copy(v_sb[sk][:, 65:129], tp[:, 64:128])
                    nc.vector.tensor_copy(v_sb[sk][:, 64:65], onesc_sb)
                    nc.vector.tensor_copy(v_sb[sk][:, 129:130], onesc_sb)

                # QT projection + rope, per e-tile (head pair)
                for j in range(NPAIR):
                    qp = qt_ps.tile([128, SC], F32, tag="qt")
                    for t in range(DT):
                        nc.tensor.matmul(
                            qp,
                            wq_sb[:, t, bass.ts(j, 128)],
                            xt_t[:, t, :],
                            start=(t == 0),
                            stop=(t == DT - 1),
                        )
                    rope(qt_sb[j], qp, cs, "qraw")

        # ---------------- Phase 2 + 3: attention + o_proj ------------------------
        with (
            tc.tile_pool(name="wop", bufs=1) as wop,
            tc.tile_pool(name="attnp", bufs=1) as attnp,
            tc.tile_pool(name="expp", bufs=6) as expp,
            tc.tile_pool(name="recp", bufs=4) as recp,
            tc.tile_pool(name="ostg", bufs=3) as ostg,
            tc.tile_pool(name="sc_ps", bufs=2, space="PSUM") as sc_ps,
            tc.tile_pool(name="av_ps", bufs=1, space="PSUM") as av_ps,
            tc.tile_pool(name="mi_ps", bufs=2, space="PSUM") as mi_ps,
        ):
            wo_sb = wop.tile([128, ET, D], BF16, tag="wo")
            nc.scalar.dma_start(out=wo_sb, in_=wo.rearrange("(t p) d -> p t d", p=128))
            attn_sb = [attnp.tile([128, S], BF16, tag=f"at{j}", name=f"at{j}") for j in range(NPAIR)]

            pending = []

            def make_normalize(attn_slice, den, half):
                def run():
                    rec32 = recp.tile([1, SC], F32, tag="rec32", name="rec32")
                    nc.vector.reciprocal_approx_fast(rec32, den)
                    rec = recp.tile([1, SC], F32R, tag="rec")
                    nc.vector.tensor_copy(rec, rec32)
                    bp = mi_ps.tile([128, SC], F32, tag="mi", name="bp")
                    nc.tensor.matmul(bp, ones1_sb, rec, start=True, stop=True)
                    nc.vector.tensor_mul(
                        attn_slice, attn_slice, bp[bass.ds(64 * half, 64), :]
                    )

                return run

            def attention(cs, j, pe_filler=None):
                """Head pair j (local heads j on kv0, j+4 on kv1), sq chunk cs."""
                sq = bass.ts(cs, SC)
                av_a = av_ps.tile([65, SC], F32, tag="ava")
                av_b = av_ps.tile([65, SC], F32, tag="avb")
                sc_t = [None, None]
                exp_t = [None] * SKT

                def scores(jj):
                    t = sc_ps.tile([128, 2 * SC], F32, tag="sc", name="sc")
                    sc_t[jj % 2] = t
                    nc.tensor.matmul(
                        t[:, 0:SC],
                        kt_sb[0:64, bass.ts(jj, 128)],
                        qt_sb[j][0:64, sq],
                        start=True,
                        stop=True,
                        tile_position=(0, 0),
                    )
                    nc.tensor.matmul(
                        t[:, SC : 2 * SC],
                        kt_sb[64:128, bass.ts(jj, 128)],
                        qt_sb[j][64:128, sq],
                        start=True,
                        stop=True,
                        tile_position=(64, 0),
                    )

                def av(t):
                    nc.tensor.matmul(
                        av_a,
                        v_sb[t][:, 0:65],
                        exp_t[t][:, 0:SC],
                        start=(t == 0),
                        stop=(t == SKT - 1),
                    )
                    nc.tensor.matmul(
                        av_b,
                        v_sb[t][:, 65:130],
                        exp_t[t][:, SC : 2 * SC],
                        start=(t == 0),
                        stop=(t == SKT - 1),
                    )

                # AV runs one iteration behind its exp so the PE stream never
                # blocks on ACT latency (kills the per-pair HAM re-throttle).
                scores(0)
                for jj in range(SKT):
                    et = expp.tile([128, 2 * SC], BF16, tag="exp")
                    exp_t[jj] = et
                    nc.scalar.activation(et, sc_t[jj % 2], AF.Exp, scale=SCALE)
                    if jj + 1 < SKT:
                        scores(jj + 1)
                    if jj >= 2:
                        av(jj - 2)
                    if jj < 2 and pending:
                        pending.pop(0)()
                    if pe_filler is not None:
                        pe_filler(jj)
                av(SKT - 2)
                av(SKT - 1)

                # Get raw attn + denominator out of PSUM fast (releases av
                # banks); push the normalization to run during the next pair.
                for half, av in ((0, av_a), (1, av_b)):
                    attn_slice = attn_sb[j][bass.ds(64 * half, 64), sq]
                    nc.vector.tensor_copy(attn_slice, av[0:64, :])
                    den = recp.tile([1, SC], F32, tag="den", name="den")
                    nc.vector.tensor_copy(den, av[64:65, :])
                    pending.append(make_normalize(attn_slice, den, half))

            def make_oproj_filler(cs, st_local):
                """Returns a per-jj filler that emits o_proj work for sq-subtile
                st_local of chunk cs, one dm-chunk (4 mms + copy) at a time,
                spread over the 16-iteration attention j-loop."""
                st = cs * 4 + st_local
                ot = ostg.tile([128, D], F32, tag="ostg", name="ostg")
                state = {"mc": 0}

                def filler(jj):
                    # emit one dm-chunk every 4 j-iterations (4 chunks over
                    # 16 j); slot %4==1 keeps the DVE ot-copy away from the
                    # pair-end av->attn copies that gate the next pair's AV.
                    if jj % 4 != 1 or state["mc"] >= D // SC:
                        return
                    mc = state["mc"]
                    state["mc"] += 1
                    op = mi_ps.tile([128, SC], F32, tag="mi", name="op")
                    for t in range(ET):
                        nc.tensor.matmul(
                            op,
                            attn_sb[t][:, bass.ts(st, 128)],
                            wo_sb[:, t, bass.ts(mc, SC)],
                            start=(t == 0),
                            stop=(t == ET - 1),
                        )
                    nc.vector.tensor_copy(ot[:, bass.ts(mc, SC)], op)
                    nc.sync.dma_start(
                        out=out[bass.ts(st, 128), bass.ts(mc, SC)],
                        in_=ot[:, bass.ts(mc, SC)],
                    )

                return filler

            for cs in range(NSC):
                for j in range(NPAIR):
                    filler = make_oproj_filler(cs - 1, j) if cs > 0 else None
                    attention(cs, j, pe_filler=filler)
            while pending:
                pending.pop(0)()
            for j in range(NPAIR):
                filler = make_oproj_filler(NSC - 1, j)
                for jj in range(SKT):
                    filler(jj)

    nc.compile()
    return nc


_PROGRAM = None


def _get_program():
    global _PROGRAM
    if _PROGRAM is None:
        _PROGRAM = build_program()
    return _PROGRAM


def _rope_tables():
    inv_freq = 1.0 / (ROPE_BASE ** (np.arange(0, HD, 2, dtype=np.float32) / HD))
    t = np.arange(S, dtype=np.float32)
    freqs = np.outer(t, inv_freq)  # [S, 32]
    emb = np.concatenate([freqs, freqs], axis=-1)  # [S, 64]
    return np.cos(emb).astype(np.float32), np.sin(emb).astype(np.float32)


def _host_constants():
    cos_t, sin_t = _rope_tables()  # [S, 64]
    idx = np.arange(128) % HD
    cosd = np.ascontiguousarray(cos_t[:, idx].T).astype(NP_BF16)  # [128, S]
    sind = np.ascontiguousarray(sin_t[:, idx].T).astype(NP_BF16)

    # rmat[k, m]: rot(q)[m] = -q[m+32] for m%64<32, +q[m-32] for m%64>=32
    rmat = np.zeros((128, 128), np.float32)
    for m in range(128):
        if m % HD < 32:
            rmat[m + 32, m] = -1.0
        else:
            rmat[m - 32, m] = 1.0
    ident = np.eye(128, dtype=NP_BF16)
    ones1 = np.ones((1, 128), np.float32)
    onesc = np.ones((128, 1), NP_BF16)
    return cosd, sind, rmat, ident, ones1, onesc


def _core_inputs(x, Wq, Wk, Wv, Wo, consts, xt_by_batch, core):
    b, g = divmod(core, 4)
    cosd, sind, rmat, ident, ones1, onesc = consts

    wq_c = np.empty((D, E), NP_BF16)
    wo_c = np.empty((E, D), NP_BF16)
    for j in range(NPAIR):
        ha = 8 * g + j  # global head, kv-head 2g
        hb = 8 * g + j + 4  # global head, kv-head 2g+1
        wq_c[:, j * 128 : j * 128 + 64] = Wq[:, ha * HD : (ha + 1) * HD]
        wq_c[:, j * 128 + 64 : (j + 1) * 128] = Wq[:, hb * HD : (hb + 1) * HD]
        wo_c[j * 128 : j * 128 + 64, :] = Wo[ha * HD : (ha + 1) * HD, :]
        wo_c[j * 128 + 64 : (j + 1) * 128, :] = Wo[hb * HD : (hb + 1) * HD, :]
    wk_c = Wk[:, 2 * g * HD : 2 * g * HD + KVW].astype(NP_BF16)
    wv_c = Wv[:, 2 * g * HD : 2 * g * HD + KVW].astype(NP_BF16)

    return {
        "xt": xt_by_batch[b],
        "wq": wq_c,
        "wk": wk_c,
        "wv": wv_c,
        "wo": wo_c,
        "cosd": cosd,
        "sind": sind,
        "rmat": rmat,
        "ident": ident,
        "ones1": ones1,
        "onesc": onesc,
    }


def make_in_maps(x, Wq, Wk, Wv, Wo):
    consts = _host_constants()
    xt_by_batch = [np.ascontiguousarray(x[b].T).astype(NP_BF16) for b in range(B)]
    return [
        _core_inputs(x, Wq, Wk, Wv, Wo, consts, xt_by_batch, c)
        for c in range(N_CORES)
    ]


def kernel(x, Wq, Wk, Wv, Wo, _trace=False, _trace_kwargs=None):
    x = np.asarray(x, np.float32)
    Wq = np.asarray(Wq, np.float32)
    Wk = np.asarray(Wk, np.float32)
    Wv = np.asarray(Wv, np.float32)
    Wo = np.asarray(Wo, np.float32)

    nc = _get_program()
    in_maps = make_in_maps(x, Wq, Wk, Wv, Wo)
    res = bass_utils.run_bass_kernel_spmd(
        nc,
        in_maps,
        core_ids=list(range(N_CORES)),
        trace=_trace,
        **(_trace_kwargs or {}),
    )
    outs = [r["out"] for r in res.results]
    full = np.empty((B, S, D), np.float32)
    for b in range(B):
        full[b] = outs[4 * b] + outs[4 * b + 1] + outs[4 * b + 2] + outs[4 * b + 3]
    if _trace:
        return full, res
    return full

